# revision 28
# baseline (speedup 1.0000x reference)
"""Trainium2 Bass kernel for nn_AttentionLayer (B=4, L=S=2048, D=1024, H=16).

Sharding: 8 cores = (batch b in 0..3) x (head-group g in 0..1); each core
handles one batch and 8 heads (512 of the 1024 q/k/v/o channels).

Per-core device kernel (all matmuls bf16, fp32 PSUM accumulation):
  1. QKV projections producing head-transposed activations:
       q^T, k^T : [E-block(128) partitions, L]   (e on partitions)
       v        : [s partitions, per-head 64 cols + a ones column]
     (bk is dropped: softmax is invariant to a per-key constant shift q.bk;
      bv and bo are added on the host: softmax rows sum to 1, so
      out += Wo @ bv + bo exactly.)
  2. Causal attention per head, scores kept transposed (P^T = [s, l]):
       S^T = (K^T)^T-stationary @ Q^T-moving, exp via ScalarE (scale=1/8
       folded in), triangular mask on diagonal blocks via VectorE,
       O^T = V_ext^T @ P^T with V_ext = [V | 1] so PSUM row 64 accumulates
       the softmax denominator. Divide via GpSimd partition-broadcast of
       the reciprocal row + VectorE multiply.
  3. Output projection: out[l, d] = sum_e O^T[e, l-block] @ Wo^T[e, d].

Host: shards/transposes/casts inputs, runs the SPMD kernel on 8 cores,
sums the two head-group partial outputs per batch, adds Wo@bv + bo.
"""

import numpy as np
import ml_dtypes

B, L, S, D, H, E = 4, 2048, 2048, 1024, 16, 64
NCORES = 8
GROUPS = 2                 # head-groups (tensor-parallel dimension)
HC = H // GROUPS           # heads per core = 8
EC = HC * E                # channels per core = 512
CH = 1024                  # attention l-chunk size

_BF16 = ml_dtypes.bfloat16


def build(L=L, S=S, D=D, HC=HC, E=E, CH=CH, debug=False, dump=False):
    import concourse.bass as bass
    import concourse.mybir as mybir
    import concourse.tile as tile
    from concourse import bacc
    from concourse.masks import make_upper_triangular

    f32 = mybir.dt.float32
    bf16 = mybir.dt.bfloat16

    EC = HC * E
    KD = D // 128           # k-tiles over d
    MB = EC // 128          # e-blocks (128 wide) per core
    LB = L // 128           # l/s blocks of 128
    NJ = L // CH            # l-chunks
    SBC = CH // 128         # s-blocks per l-chunk
    HPB = 128 // E          # heads per e-block = 2

    nc = bacc.Bacc(None, target_bir_lowering=False, debug=debug)

    qT = nc.dram_tensor("qT", [D, L], bf16, kind="ExternalInput")
    kT = nc.dram_tensor("kT", [D, S], bf16, kind="ExternalInput")
    vT = nc.dram_tensor("vT", [D, S], bf16, kind="ExternalInput")
    wq = nc.dram_tensor("wq", [D, EC], bf16, kind="ExternalInput")
    wk = nc.dram_tensor("wk", [D, EC], bf16, kind="ExternalInput")
    wv = nc.dram_tensor("wv", [D, EC], bf16, kind="ExternalInput")
    wo = nc.dram_tensor("wo", [EC, D], bf16, kind="ExternalInput")
    bq = nc.dram_tensor("bq", [128, MB], f32, kind="ExternalInput")
    out = nc.dram_tensor("out", [L, D], f32, kind="ExternalOutput")

    scale = 1.0 / float(np.sqrt(E))

    with tile.TileContext(nc) as tc:
        with (
            tc.tile_pool(name="persist", bufs=1) as pp,
            tc.tile_pool(name="weights", bufs=1) as wp,
            tc.tile_pool(name="inputs", bufs=3 * KD) as ip,
            tc.tile_pool(name="work", bufs=3) as kp,
            tc.tile_pool(name="divp", bufs=2) as dp,
        ):
            # ---- persistent SBUF tensors ----
            q_sb = pp.tile([128, MB, L], bf16, tag="q_sb")
            k_sb = pp.tile([128, HC, S], bf16, tag="k_sb")
            v_sb = pp.tile([128, S // 128, HC, E + 1], bf16, tag="v_sb")
            o_sb = pp.tile([128, MB, L], bf16, tag="o_sb")
            tri = pp.tile([128, 128], bf16, tag="tri")
            bq_t = pp.tile([128, MB], f32, tag="bq_t")

            make_upper_triangular(nc, tri[:, :], val=1.0, diag=True)
            for hh in range(HPB):
                other = (1 - hh) * E
                nc.vector.memset(k_sb[other : other + E, hh::HPB, :], 0.0)
            nc.sync.dma_start(bq_t[:], bq[:])
            # ones column for the softmax denominator (PSUM partition E=64)
            nc.vector.memset(v_sb[:, :, :, E : E + 1], 1.0)

            # ---- weights ----
            wq_sb = wp.tile([128, KD, EC], bf16, tag="wq")
            wk_sb = wp.tile([128, KD, EC], bf16, tag="wk")
            wv_sb = wp.tile([128, KD, EC], bf16, tag="wv")
            wo_sb = wp.tile([128, MB, D], bf16, tag="wo")
            for k in range(KD):
                nc.sync.dma_start(wv_sb[:, k, :], wv[128 * k : 128 * (k + 1), :])

            # ---- projections ----
            # Inputs staged as [128, 512] quarter-tiles, loaded in the exact
            # order each projection consumes them (n-outer loops), so compute
            # starts ~2 MB into the DMA stream and never re-starves.
            IQ = 512
            NQ = S // IQ

            def _ld(dram, k, qq, split=False):
                tq = ip.tile([128, IQ], bf16, tag="inq",
                             name=f"in_{dram.name}_{k}_{qq}")
                if split:  # two partition-half DMAs -> two queues in parallel
                    nc.sync.dma_start(
                        tq[0:64, :],
                        dram[128 * k : 128 * k + 64, IQ * qq : IQ * (qq + 1)],
                    )
                    nc.sync.dma_start(
                        tq[64:128, :],
                        dram[128 * k + 64 : 128 * (k + 1), IQ * qq : IQ * (qq + 1)],
                    )
                else:
                    nc.sync.dma_start(
                        tq[:], dram[128 * k : 128 * (k + 1), IQ * qq : IQ * (qq + 1)]
                    )
                return tq

            with tc.tile_pool(name="psA", bufs=8, space="PSUM") as psA:
                vt = {}
                qt = {}
                kt = {}
                for k in range(KD):
                    vt[k, 0] = _ld(vT, k, 0, split=True)
                for k in range(KD):
                    # split wv across two queues as well: these 2 MB gate the
                    # very first matmul
                    nc.sync.dma_start(
                        wv_sb[0:64, k, :], wv[128 * k : 128 * k + 64, :]
                    )
                    nc.sync.dma_start(
                        wv_sb[64:128, k, :], wv[128 * k + 64 : 128 * (k + 1), :]
                    )
                if NQ > 1:
                    for k in range(KD):
                        vt[k, 1] = _ld(vT, k, 1)
                for k in range(KD):
                    nc.sync.dma_start(wq_sb[:, k, :], wq[128 * k : 128 * (k + 1), :])

                # v projection: v[s, e] = values @ Wv.T (natural layout)
                for qq in range(NQ):
                    if qq + 2 < NQ:
                        for k in range(KD):
                            vt[k, qq + 2] = _ld(vT, k, qq + 2)
                    for k in range(KD):
                        qt[k, qq] = _ld(qT, k, qq)
                    if qq == NQ - 1:
                        for k in range(KD):
                            nc.sync.dma_start(
                                wk_sb[:, k, :], wk[128 * k : 128 * (k + 1), :]
                            )
                        for k in range(MB):
                            nc.sync.dma_start(
                                wo_sb[:, k, :], wo[128 * k : 128 * (k + 1), :]
                            )
                    for sb in range(4 * qq, 4 * (qq + 1)):
                        ps = psA.tile([128, EC], f32, tag="ps", name=f"vp{sb}")
                        for k in range(KD):
                            nc.tensor.matmul(
                                ps[:, :],
                                vt[k, qq][:, 128 * (sb % 4) : 128 * (sb % 4 + 1)],
                                wv_sb[:, k, :],
                                start=(k == 0),
                                stop=(k == KD - 1),
                            )
                        nc.scalar.activation(
                            v_sb[:, sb, :, 0:E],
                            ps[:, :].rearrange("p (h e) -> p h e", h=HC),
                            mybir.ActivationFunctionType.Copy,
                        )

                # q projection: q^T[e, l] (+ per-partition bias bq)
                for n in range(NQ):
                    for k in range(KD):
                        kt[k, n] = _ld(kT, k, n)
                    for m in range(MB):
                        ps = psA.tile([128, 512], f32, tag="ps", name=f"qp{m}_{n}")
                        for k in range(KD):
                            nc.tensor.matmul(
                                ps[:, :],
                                wq_sb[:, k, 128 * m : 128 * (m + 1)],
                                qt[k, n][:, :],
                                start=(k == 0),
                                stop=(k == KD - 1),
                            )
                        nc.vector.tensor_scalar_add(
                            q_sb[:, m, 512 * n : 512 * (n + 1)],
                            ps[:, :],
                            bq_t[:, m : m + 1],
                        )

                # k projection: zero-padded per-head k^T[e, s] (no bias)
                for n in range(NQ):
                    for m in range(MB):
                        ps = psA.tile([128, 512], f32, tag="ps", name=f"kp{m}_{n}")
                        for k in range(KD):
                            nc.tensor.matmul(
                                ps[:, :],
                                wk_sb[:, k, 128 * m : 128 * (m + 1)],
                                kt[k, n][:, :],
                                start=(k == 0),
                                stop=(k == KD - 1),
                            )
                        for hh in range(HPB):
                            nc.vector.tensor_copy(
                                k_sb[hh * E : (hh + 1) * E,
                                     HPB * m + hh, 512 * n : 512 * (n + 1)],
                                ps[hh * E : (hh + 1) * E, :],
                            )

            # ---- attention ----
            with (
                tc.tile_pool(name="psS", bufs=2, space="PSUM") as psS,
                tc.tile_pool(name="psO", bufs=2, space="PSUM") as psO,
            ):
                for j in range(NJ):
                    for h in range(HC):
                        m = h // HPB
                        poff = (h % HPB) * E
                        ops = psO.tile([128, CH], f32, tag="ops",
                                       name=f"ops{h}_{j}")
                        nsb = SBC * (j + 1)  # s-blocks touching this chunk
                        for i in range(nsb):
                            col0 = max(0, 128 * i - CH * j)
                            bl = []
                            c = col0
                            while c < CH:
                                n = min(512 - (c % 512), CH - c)
                                bl.append((c, n))
                                c += n
                            sps = psS.tile([128, CH], f32, tag="sps",
                                           name=f"sps{h}_{i}_{j}")
                            for (c, n) in bl:
                                nc.tensor.matmul(
                                    sps[:, c : c + n],
                                    k_sb[:, h, 128 * i : 128 * (i + 1)],
                                    q_sb[:, m, CH * j + c : CH * j + c + n],
                                    start=True,
                                    stop=True,
                                )
                            p_t = kp.tile([128, CH], bf16, tag="p_t",
                                          name=f"pt{h}_{i}_{j}")
                            nc.scalar.activation(
                                p_t[:, col0:CH],
                                sps[:, col0:CH],
                                mybir.ActivationFunctionType.Exp,
                                scale=scale,
                            )
                            if i >= SBC * j:  # diagonal block: causal mask
                                nc.vector.tensor_mul(
                                    p_t[:, col0 : col0 + 128],
                                    p_t[:, col0 : col0 + 128],
                                    tri[:, :],
                                )
                            for (c, n) in bl:
                                be = min(CH, 512 * (c // 512) + 512)
                                i_last = (be + CH * j) // 128 - 1
                                nc.tensor.matmul(
                                    ops[0 : E + 1, c : c + n],
                                    v_sb[:, i, h, :],
                                    p_t[:, c : c + n],
                                    start=(i == 0),
                                    stop=(i == i_last),
                                )
                        # softmax denominator: PSUM row E=64 of ops
                        rs = dp.tile([128, CH], f32, tag="rs", bufs=2)
                        nc.vector.tensor_copy(rs[E : E + 1, :], ops[E : E + 1, :])
                        rs0 = dp.tile([128, CH], f32, tag="rs0", bufs=2)
                        # custom GpSimd/DVE ops need base-partition-0 operands
                        # on HW: DMA the sum row to partition 0, broadcast it,
                        # then take the approx reciprocal on the [0:E] block.
                        nc.sync.dma_start(rs0[0:1, :], rs[E : E + 1, :])
                        rr = dp.tile([128, CH], f32, tag="rr")
                        nc.gpsimd.partition_broadcast(
                            rr[0:E, :], rs0[0:1, :], channels=E
                        )
                        nc.vector.reciprocal_approx_fast(rr[0:E, :], rr[0:E, :])
                        o_tmp = dp.tile([128, CH], bf16, tag="o_tmp")
                        nc.vector.tensor_mul(
                            o_tmp[0:E, :], ops[0:E, :], rr[0:E, :]
                        )
                        nc.sync.dma_start(
                            o_sb[poff : poff + E, m, CH * j : CH * (j + 1)],
                            o_tmp[0:E, :],
                        )

                # ---- output projection (shares the "sps" PSUM slots so its
                # matmuls backfill the tail of the attention pipeline) ----
                DC = min(512, D)
                NDC = D // DC
                for lb in range(LB):
                    for dc in range(NDC):
                        pso = psS.tile([128, DC], f32, tag="sps",
                                       name=f"op{lb}_{dc}")
                        for k in range(MB):
                            nc.tensor.matmul(
                                pso[:, :],
                                o_sb[:, k, 128 * lb : 128 * (lb + 1)],
                                wo_sb[:, k, DC * dc : DC * (dc + 1)],
                                start=(k == 0),
                                stop=(k == MB - 1),
                            )
                        ot = kp.tile([128, DC], f32, tag="ot", name=f"ot{lb}_{dc}")
                        nc.vector.tensor_copy(ot[:, :], pso[:, :])
                        nc.sync.dma_start(
                            out[128 * lb : 128 * (lb + 1), DC * dc : DC * (dc + 1)],
                            ot[:, :],
                        )

            if dump:
                dq = nc.dram_tensor("dq", [128, MB, L], bf16, kind="ExternalOutput")
                dk = nc.dram_tensor("dk", [128, HC, S], bf16, kind="ExternalOutput")
                dv = nc.dram_tensor(
                    "dv", [128, S // 128, HC, E + 1], bf16, kind="ExternalOutput"
                )
                do = nc.dram_tensor("do", [128, MB, L], bf16, kind="ExternalOutput")
                nc.sync.dma_start(dq[:], q_sb[:])
                nc.sync.dma_start(dk[:], k_sb[:])
                nc.sync.dma_start(dv[:], v_sb[:])
                nc.sync.dma_start(do[:], o_sb[:])


    nc.compile()
    return nc


def _prep_inputs(queries, keys, values, Wq, bq, Wk, Wv, Wo):
    """Build the 8 per-core input maps (host-side shard + transpose + cast)."""
    MB = EC // 128
    in_maps = []
    qT = [np.ascontiguousarray(queries[b].T.astype(_BF16)) for b in range(B)]
    kT = [np.ascontiguousarray(keys[b].T.astype(_BF16)) for b in range(B)]
    vT = [np.ascontiguousarray(values[b].T.astype(_BF16)) for b in range(B)]
    wqs, wks, wvs, wos, bqs = [], [], [], [], []
    for g in range(GROUPS):
        sl = slice(g * EC, (g + 1) * EC)
        wqs.append(np.ascontiguousarray(Wq[sl, :].T.astype(_BF16)))
        wks.append(np.ascontiguousarray(Wk[sl, :].T.astype(_BF16)))
        wvs.append(np.ascontiguousarray(Wv[sl, :].T.astype(_BF16)))
        wos.append(np.ascontiguousarray(Wo[:, sl].T.astype(_BF16)))
        bqs.append(
            np.ascontiguousarray(
                bq[sl].astype(np.float32).reshape(MB, 128).T
            )
        )
    for c in range(NCORES):
        b, g = c // GROUPS, c % GROUPS
        in_maps.append(
            {
                "qT": qT[b], "kT": kT[b], "vT": vT[b],
                "wq": wqs[g], "wk": wks[g], "wv": wvs[g],
                "wo": wos[g], "bq": bqs[g],
            }
        )
    return in_maps


_NC_CACHE = {}


def kernel(queries, keys, values, attn_mask, Wq, bq, Wk, bk, Wv, bv, Wo, bo,
           _trace=False):
    from concourse.bass_utils import run_bass_kernel_spmd

    queries = np.asarray(queries, np.float32)
    keys = np.asarray(keys, np.float32)
    values = np.asarray(values, np.float32)
    Wq, Wk, Wv, Wo = (np.asarray(a, np.float32) for a in (Wq, Wk, Wv, Wo))
    bq, bk, bv, bo = (np.asarray(a, np.float32) for a in (bq, bk, bv, bo))

    if "nc" not in _NC_CACHE:
        _NC_CACHE["nc"] = build()
    nc = _NC_CACHE["nc"]

    in_maps = _prep_inputs(queries, keys, values, Wq, bq, Wk, Wv, Wo)
    res = run_bass_kernel_spmd(
        nc, in_maps, core_ids=list(range(NCORES)), trace=_trace
    )
    _NC_CACHE["last_results"] = res

    out = np.zeros((B, L, D), np.float32)
    for c in range(NCORES):
        out[c // GROUPS] += res.results[c]["out"]
    # bv exits through the (row-sum-1) softmax as Wo @ bv; bo is direct.
    out += (Wo @ bv + bo)[None, None, :]
    return out


# revision 29
# speedup vs baseline: 1.0194x; 1.0194x over previous
"""Trainium2 Bass kernel for nn_AttentionLayer (B=4, L=S=2048, D=1024, H=16).

Sharding: 8 cores = (batch b in 0..3) x (head-group g in 0..1); each core
handles one batch and 8 heads (512 of the 1024 q/k/v/o channels).

Per-core device kernel (all matmuls bf16, fp32 PSUM accumulation):
  1. QKV projections producing head-transposed activations:
       q^T, k^T : [E-block(128) partitions, L]   (e on partitions)
       v        : [s partitions, per-head 64 cols + a ones column]
     (bk is dropped: softmax is invariant to a per-key constant shift q.bk;
      bv and bo are added on the host: softmax rows sum to 1, so
      out += Wo @ bv + bo exactly.)
  2. Causal attention per head, scores kept transposed (P^T = [s, l]):
       S^T = (K^T)^T-stationary @ Q^T-moving, exp via ScalarE (scale=1/8
       folded in), triangular mask on diagonal blocks via VectorE,
       O^T = V_ext^T @ P^T with V_ext = [V | 1] so PSUM row 64 accumulates
       the softmax denominator. Divide via GpSimd partition-broadcast of
       the reciprocal row + VectorE multiply.
  3. Output projection: out[l, d] = sum_e O^T[e, l-block] @ Wo^T[e, d].

Host: shards/transposes/casts inputs, runs the SPMD kernel on 8 cores,
sums the two head-group partial outputs per batch, adds Wo@bv + bo.
"""

import numpy as np
import ml_dtypes

B, L, S, D, H, E = 4, 2048, 2048, 1024, 16, 64
NCORES = 8
GROUPS = 2                 # head-groups (tensor-parallel dimension)
HC = H // GROUPS           # heads per core = 8
EC = HC * E                # channels per core = 512
CH = 1024                  # attention l-chunk size

_BF16 = ml_dtypes.bfloat16


def build(L=L, S=S, D=D, HC=HC, E=E, CH=CH, debug=False, dump=False):
    import concourse.bass as bass
    import concourse.mybir as mybir
    import concourse.tile as tile
    from concourse import bacc
    from concourse.masks import make_upper_triangular

    f32 = mybir.dt.float32
    bf16 = mybir.dt.bfloat16

    EC = HC * E
    KD = D // 128           # k-tiles over d
    MB = EC // 128          # e-blocks (128 wide) per core
    LB = L // 128           # l/s blocks of 128
    NJ = L // CH            # l-chunks
    SBC = CH // 128         # s-blocks per l-chunk
    HPB = 128 // E          # heads per e-block = 2

    nc = bacc.Bacc(None, target_bir_lowering=False, debug=debug)

    qT = nc.dram_tensor("qT", [D, L], bf16, kind="ExternalInput")
    kT = nc.dram_tensor("kT", [D, S], bf16, kind="ExternalInput")
    vT = nc.dram_tensor("vT", [D, S], bf16, kind="ExternalInput")
    wq = nc.dram_tensor("wq", [D, EC], bf16, kind="ExternalInput")
    wk = nc.dram_tensor("wk", [D, EC], bf16, kind="ExternalInput")
    wv = nc.dram_tensor("wv", [D, EC], bf16, kind="ExternalInput")
    wo = nc.dram_tensor("wo", [EC, D], bf16, kind="ExternalInput")
    bq = nc.dram_tensor("bq", [128, MB], f32, kind="ExternalInput")
    out = nc.dram_tensor("out", [L, D], f32, kind="ExternalOutput")

    scale = 1.0 / float(np.sqrt(E))

    with tile.TileContext(nc) as tc:
        with (
            tc.tile_pool(name="persist", bufs=1) as pp,
            tc.tile_pool(name="weights", bufs=1) as wp,
            tc.tile_pool(name="inputs", bufs=3 * KD) as ip,
            tc.tile_pool(name="work", bufs=3) as kp,
            tc.tile_pool(name="divp", bufs=2) as dp,
        ):
            # ---- persistent SBUF tensors ----
            q_sb = pp.tile([128, MB, L], bf16, tag="q_sb")
            k_sb = pp.tile([128, HC, S], bf16, tag="k_sb")
            v_sb = pp.tile([128, S // 128, HC, E + 1], bf16, tag="v_sb")
            o_sb = pp.tile([128, MB, L], bf16, tag="o_sb")
            tri = pp.tile([128, 128], bf16, tag="tri")
            bq_t = pp.tile([128, MB], f32, tag="bq_t")

            make_upper_triangular(nc, tri[:, :], val=1.0, diag=True)
            for hh in range(HPB):
                other = (1 - hh) * E
                nc.vector.memset(k_sb[other : other + E, hh::HPB, :], 0.0)
            nc.sync.dma_start(bq_t[:], bq[:])
            # ones column for the softmax denominator (PSUM partition E=64)
            nc.vector.memset(v_sb[:, :, :, E : E + 1], 1.0)

            # ---- weights ----
            wq_sb = wp.tile([128, KD, EC], bf16, tag="wq")
            wk_sb = wp.tile([128, KD, EC], bf16, tag="wk")
            wv_sb = wp.tile([128, KD, EC], bf16, tag="wv")
            wo_sb = wp.tile([128, MB, D], bf16, tag="wo")
            for k in range(KD):
                nc.sync.dma_start(wv_sb[:, k, :], wv[128 * k : 128 * (k + 1), :])

            # ---- projections ----
            # Inputs staged as [128, 512] quarter-tiles, loaded in the exact
            # order each projection consumes them (n-outer loops), so compute
            # starts ~2 MB into the DMA stream and never re-starves.
            IQ = 512
            NQ = S // IQ

            def _ld(dram, k, qq, split=False):
                tq = ip.tile([128, IQ], bf16, tag="inq",
                             name=f"in_{dram.name}_{k}_{qq}")
                if split:  # two partition-half DMAs -> two queues in parallel
                    nc.sync.dma_start(
                        tq[0:64, :],
                        dram[128 * k : 128 * k + 64, IQ * qq : IQ * (qq + 1)],
                    )
                    nc.sync.dma_start(
                        tq[64:128, :],
                        dram[128 * k + 64 : 128 * (k + 1), IQ * qq : IQ * (qq + 1)],
                    )
                else:
                    nc.sync.dma_start(
                        tq[:], dram[128 * k : 128 * (k + 1), IQ * qq : IQ * (qq + 1)]
                    )
                return tq

            with tc.tile_pool(name="psA", bufs=8, space="PSUM") as psA:
                vt = {}
                qt = {}
                kt = {}
                for k in range(KD):
                    vt[k, 0] = _ld(vT, k, 0)
                for k in range(KD):
                    nc.sync.dma_start(wv_sb[:, k, :], wv[128 * k : 128 * (k + 1), :])
                if NQ > 1:
                    for k in range(KD):
                        vt[k, 1] = _ld(vT, k, 1)
                for k in range(KD):
                    nc.sync.dma_start(wq_sb[:, k, :], wq[128 * k : 128 * (k + 1), :])

                # v projection: v[s, e] = values @ Wv.T (natural layout)
                for qq in range(NQ):
                    if qq + 2 < NQ:
                        for k in range(KD):
                            vt[k, qq + 2] = _ld(vT, k, qq + 2)
                    for k in range(KD):
                        qt[k, qq] = _ld(qT, k, qq)
                    if qq == NQ - 1:
                        for k in range(KD):
                            nc.sync.dma_start(
                                wk_sb[:, k, :], wk[128 * k : 128 * (k + 1), :]
                            )
                        for k in range(MB):
                            nc.sync.dma_start(
                                wo_sb[:, k, :], wo[128 * k : 128 * (k + 1), :]
                            )
                    for sb in range(4 * qq, 4 * (qq + 1)):
                        ps = psA.tile([128, EC], f32, tag="ps", name=f"vp{sb}")
                        for k in range(KD):
                            nc.tensor.matmul(
                                ps[:, :],
                                vt[k, qq][:, 128 * (sb % 4) : 128 * (sb % 4 + 1)],
                                wv_sb[:, k, :],
                                start=(k == 0),
                                stop=(k == KD - 1),
                            )
                        nc.scalar.activation(
                            v_sb[:, sb, :, 0:E],
                            ps[:, :].rearrange("p (h e) -> p h e", h=HC),
                            mybir.ActivationFunctionType.Copy,
                        )

                # q projection: q^T[e, l] (+ per-partition bias bq)
                for n in range(NQ):
                    for k in range(KD):
                        kt[k, n] = _ld(kT, k, n)
                    for m in range(MB):
                        ps = psA.tile([128, 512], f32, tag="ps", name=f"qp{m}_{n}")
                        for k in range(KD):
                            nc.tensor.matmul(
                                ps[:, :],
                                wq_sb[:, k, 128 * m : 128 * (m + 1)],
                                qt[k, n][:, :],
                                start=(k == 0),
                                stop=(k == KD - 1),
                            )
                        nc.vector.tensor_scalar_add(
                            q_sb[:, m, 512 * n : 512 * (n + 1)],
                            ps[:, :],
                            bq_t[:, m : m + 1],
                        )

                # k projection: zero-padded per-head k^T[e, s] (no bias)
                for n in range(NQ):
                    for m in range(MB):
                        ps = psA.tile([128, 512], f32, tag="ps", name=f"kp{m}_{n}")
                        for k in range(KD):
                            nc.tensor.matmul(
                                ps[:, :],
                                wk_sb[:, k, 128 * m : 128 * (m + 1)],
                                kt[k, n][:, :],
                                start=(k == 0),
                                stop=(k == KD - 1),
                            )
                        for hh in range(HPB):
                            nc.vector.tensor_copy(
                                k_sb[hh * E : (hh + 1) * E,
                                     HPB * m + hh, 512 * n : 512 * (n + 1)],
                                ps[hh * E : (hh + 1) * E, :],
                            )

            # ---- attention ----
            with (
                tc.tile_pool(name="psS", bufs=2, space="PSUM") as psS,
                tc.tile_pool(name="psO", bufs=2, space="PSUM") as psO,
            ):
                for j in range(NJ):
                    for h in range(HC):
                        m = h // HPB
                        poff = (h % HPB) * E
                        ops = psO.tile([128, CH], f32, tag="ops",
                                       name=f"ops{h}_{j}")
                        nsb = SBC * (j + 1)  # s-blocks touching this chunk
                        for i in range(nsb):
                            col0 = max(0, 128 * i - CH * j)
                            bl = []
                            c = col0
                            while c < CH:
                                n = min(512 - (c % 512), CH - c)
                                bl.append((c, n))
                                c += n
                            sps = psS.tile([128, CH], f32, tag="sps",
                                           name=f"sps{h}_{i}_{j}")
                            for (c, n) in bl:
                                nc.tensor.matmul(
                                    sps[:, c : c + n],
                                    k_sb[:, h, 128 * i : 128 * (i + 1)],
                                    q_sb[:, m, CH * j + c : CH * j + c + n],
                                    start=True,
                                    stop=True,
                                )
                            p_t = kp.tile([128, CH], bf16, tag="p_t",
                                          name=f"pt{h}_{i}_{j}")
                            nc.scalar.activation(
                                p_t[:, col0:CH],
                                sps[:, col0:CH],
                                mybir.ActivationFunctionType.Exp,
                                scale=scale,
                            )
                            if i >= SBC * j:  # diagonal block: causal mask
                                nc.vector.tensor_mul(
                                    p_t[:, col0 : col0 + 128],
                                    p_t[:, col0 : col0 + 128],
                                    tri[:, :],
                                )
                            for (c, n) in bl:
                                be = min(CH, 512 * (c // 512) + 512)
                                i_last = (be + CH * j) // 128 - 1
                                nc.tensor.matmul(
                                    ops[0 : E + 1, c : c + n],
                                    v_sb[:, i, h, :],
                                    p_t[:, c : c + n],
                                    start=(i == 0),
                                    stop=(i == i_last),
                                )
                        # softmax denominator: PSUM row E=64 of ops
                        rs = dp.tile([128, CH], f32, tag="rs", bufs=2)
                        nc.vector.tensor_copy(rs[E : E + 1, :], ops[E : E + 1, :])
                        rs0 = dp.tile([128, CH], f32, tag="rs0", bufs=2)
                        # custom GpSimd/DVE ops need base-partition-0 operands
                        # on HW: DMA the sum row to partition 0, broadcast it,
                        # then take the approx reciprocal on the [0:E] block.
                        nc.sync.dma_start(rs0[0:1, :], rs[E : E + 1, :])
                        rr = dp.tile([128, CH], f32, tag="rr")
                        nc.gpsimd.partition_broadcast(
                            rr[0:E, :], rs0[0:1, :], channels=E
                        )
                        nc.vector.reciprocal_approx_fast(rr[0:E, :], rr[0:E, :])
                        o_tmp = dp.tile([128, CH], bf16, tag="o_tmp")
                        nc.vector.tensor_mul(
                            o_tmp[0:E, :], ops[0:E, :], rr[0:E, :]
                        )
                        nc.sync.dma_start(
                            o_sb[poff : poff + E, m, CH * j : CH * (j + 1)],
                            o_tmp[0:E, :],
                        )

                # ---- output projection (shares the "sps" PSUM slots so its
                # matmuls backfill the tail of the attention pipeline) ----
                DC = min(512, D)
                NDC = D // DC
                for lb in range(LB):
                    for dc in range(NDC):
                        pso = psS.tile([128, DC], f32, tag="sps",
                                       name=f"op{lb}_{dc}")
                        for k in range(MB):
                            nc.tensor.matmul(
                                pso[:, :],
                                o_sb[:, k, 128 * lb : 128 * (lb + 1)],
                                wo_sb[:, k, DC * dc : DC * (dc + 1)],
                                start=(k == 0),
                                stop=(k == MB - 1),
                            )
                        ot = kp.tile([128, DC], f32, tag="ot", name=f"ot{lb}_{dc}")
                        nc.vector.tensor_copy(ot[:, :], pso[:, :])
                        nc.sync.dma_start(
                            out[128 * lb : 128 * (lb + 1), DC * dc : DC * (dc + 1)],
                            ot[:, :],
                        )

            if dump:
                dq = nc.dram_tensor("dq", [128, MB, L], bf16, kind="ExternalOutput")
                dk = nc.dram_tensor("dk", [128, HC, S], bf16, kind="ExternalOutput")
                dv = nc.dram_tensor(
                    "dv", [128, S // 128, HC, E + 1], bf16, kind="ExternalOutput"
                )
                do = nc.dram_tensor("do", [128, MB, L], bf16, kind="ExternalOutput")
                nc.sync.dma_start(dq[:], q_sb[:])
                nc.sync.dma_start(dk[:], k_sb[:])
                nc.sync.dma_start(dv[:], v_sb[:])
                nc.sync.dma_start(do[:], o_sb[:])


    nc.compile()
    return nc


def _prep_inputs(queries, keys, values, Wq, bq, Wk, Wv, Wo):
    """Build the 8 per-core input maps (host-side shard + transpose + cast)."""
    MB = EC // 128
    in_maps = []
    qT = [np.ascontiguousarray(queries[b].T.astype(_BF16)) for b in range(B)]
    kT = [np.ascontiguousarray(keys[b].T.astype(_BF16)) for b in range(B)]
    vT = [np.ascontiguousarray(values[b].T.astype(_BF16)) for b in range(B)]
    wqs, wks, wvs, wos, bqs = [], [], [], [], []
    for g in range(GROUPS):
        sl = slice(g * EC, (g + 1) * EC)
        wqs.append(np.ascontiguousarray(Wq[sl, :].T.astype(_BF16)))
        wks.append(np.ascontiguousarray(Wk[sl, :].T.astype(_BF16)))
        wvs.append(np.ascontiguousarray(Wv[sl, :].T.astype(_BF16)))
        wos.append(np.ascontiguousarray(Wo[:, sl].T.astype(_BF16)))
        bqs.append(
            np.ascontiguousarray(
                bq[sl].astype(np.float32).reshape(MB, 128).T
            )
        )
    for c in range(NCORES):
        b, g = c // GROUPS, c % GROUPS
        in_maps.append(
            {
                "qT": qT[b], "kT": kT[b], "vT": vT[b],
                "wq": wqs[g], "wk": wks[g], "wv": wvs[g],
                "wo": wos[g], "bq": bqs[g],
            }
        )
    return in_maps


_NC_CACHE = {}


def kernel(queries, keys, values, attn_mask, Wq, bq, Wk, bk, Wv, bv, Wo, bo,
           _trace=False):
    from concourse.bass_utils import run_bass_kernel_spmd

    queries = np.asarray(queries, np.float32)
    keys = np.asarray(keys, np.float32)
    values = np.asarray(values, np.float32)
    Wq, Wk, Wv, Wo = (np.asarray(a, np.float32) for a in (Wq, Wk, Wv, Wo))
    bq, bk, bv, bo = (np.asarray(a, np.float32) for a in (bq, bk, bv, bo))

    if "nc" not in _NC_CACHE:
        _NC_CACHE["nc"] = build()
    nc = _NC_CACHE["nc"]

    in_maps = _prep_inputs(queries, keys, values, Wq, bq, Wk, Wv, Wo)
    res = run_bass_kernel_spmd(
        nc, in_maps, core_ids=list(range(NCORES)), trace=_trace
    )
    _NC_CACHE["last_results"] = res

    out = np.zeros((B, L, D), np.float32)
    for c in range(NCORES):
        out[c // GROUPS] += res.results[c]["out"]
    # bv exits through the (row-sum-1) softmax as Wo @ bv; bo is direct.
    out += (Wo @ bv + bo)[None, None, :]
    return out


# revision 31
# speedup vs baseline: 1.0389x; 1.0192x over previous
"""Trainium2 Bass kernel for nn_AttentionLayer (B=4, L=S=2048, D=1024, H=16).

Sharding: 8 cores = (batch b in 0..3) x (head-group g in 0..1); each core
handles one batch and 8 heads (512 of the 1024 q/k/v/o channels).

Per-core device kernel (all matmuls bf16, fp32 PSUM accumulation):
  1. QKV projections producing head-transposed activations:
       q^T, k^T : [E-block(128) partitions, L]   (e on partitions)
       v        : [s partitions, per-head 64 cols + a ones column]
     (bk is dropped: softmax is invariant to a per-key constant shift q.bk;
      bv and bo are added on the host: softmax rows sum to 1, so
      out += Wo @ bv + bo exactly.)
  2. Causal attention per head, scores kept transposed (P^T = [s, l]):
       S^T = (K^T)^T-stationary @ Q^T-moving, exp via ScalarE (scale=1/8
       folded in), triangular mask on diagonal blocks via VectorE,
       O^T = V_ext^T @ P^T with V_ext = [V | 1] so PSUM row 64 accumulates
       the softmax denominator. Divide via GpSimd partition-broadcast of
       the reciprocal row + VectorE multiply.
  3. Output projection: out[l, d] = sum_e O^T[e, l-block] @ Wo^T[e, d].

Host: shards/transposes/casts inputs, runs the SPMD kernel on 8 cores,
sums the two head-group partial outputs per batch, adds Wo@bv + bo.
"""

import numpy as np
import ml_dtypes

B, L, S, D, H, E = 4, 2048, 2048, 1024, 16, 64
NCORES = 8
GROUPS = 2                 # head-groups (tensor-parallel dimension)
HC = H // GROUPS           # heads per core = 8
EC = HC * E                # channels per core = 512
CH = 1024                  # attention l-chunk size

_BF16 = ml_dtypes.bfloat16


def build(L=L, S=S, D=D, HC=HC, E=E, CH=CH, debug=False, dump=False):
    import concourse.bass as bass
    import concourse.mybir as mybir
    import concourse.tile as tile
    from concourse import bacc
    from concourse.masks import make_upper_triangular

    f32 = mybir.dt.float32
    bf16 = mybir.dt.bfloat16

    EC = HC * E
    KD = D // 128           # k-tiles over d
    MB = EC // 128          # e-blocks (128 wide) per core
    LB = L // 128           # l/s blocks of 128
    NJ = L // CH            # l-chunks
    SBC = CH // 128         # s-blocks per l-chunk
    HPB = 128 // E          # heads per e-block = 2

    nc = bacc.Bacc(None, target_bir_lowering=False, debug=debug)

    qT = nc.dram_tensor("qT", [D, L], bf16, kind="ExternalInput")
    kT = nc.dram_tensor("kT", [D, S], bf16, kind="ExternalInput")
    vT = nc.dram_tensor("vT", [D, S], bf16, kind="ExternalInput")
    wq = nc.dram_tensor("wq", [D, EC], bf16, kind="ExternalInput")
    wk = nc.dram_tensor("wk", [D, EC], bf16, kind="ExternalInput")
    wv = nc.dram_tensor("wv", [D, EC], bf16, kind="ExternalInput")
    wo = nc.dram_tensor("wo", [EC, D], bf16, kind="ExternalInput")
    bq = nc.dram_tensor("bq", [128, MB], f32, kind="ExternalInput")
    out = nc.dram_tensor("out", [L, D], f32, kind="ExternalOutput")

    scale = 1.0 / float(np.sqrt(E))

    with tile.TileContext(nc) as tc:
        with (
            tc.tile_pool(name="persist", bufs=1) as pp,
            tc.tile_pool(name="weights", bufs=1) as wp,
            tc.tile_pool(name="inputs", bufs=3 * KD) as ip,
            tc.tile_pool(name="work", bufs=4) as kp,
            tc.tile_pool(name="divp", bufs=2) as dp,
        ):
            # ---- persistent SBUF tensors ----
            q_sb = pp.tile([128, MB, L], bf16, tag="q_sb")
            k_sb = pp.tile([128, HC, S], bf16, tag="k_sb")
            v_sb = pp.tile([128, S // 128, HC, E + 1], bf16, tag="v_sb")
            o_sb = pp.tile([128, MB, L], bf16, tag="o_sb")
            tri = pp.tile([128, 128], bf16, tag="tri")
            bq_t = pp.tile([128, MB], f32, tag="bq_t")

            make_upper_triangular(nc, tri[:, :], val=1.0, diag=True)
            for hh in range(HPB):
                other = (1 - hh) * E
                nc.vector.memset(k_sb[other : other + E, hh::HPB, :], 0.0)
            nc.sync.dma_start(bq_t[:], bq[:])
            # ones column for the softmax denominator (PSUM partition E=64)
            nc.vector.memset(v_sb[:, :, :, E : E + 1], 1.0)

            # ---- weights ----
            wq_sb = wp.tile([128, KD, EC], bf16, tag="wq")
            wk_sb = wp.tile([128, KD, EC], bf16, tag="wk")
            wv_sb = wp.tile([128, KD, EC], bf16, tag="wv")
            wo_sb = wp.tile([128, MB, D], bf16, tag="wo")
            for k in range(KD):
                nc.sync.dma_start(wv_sb[:, k, :], wv[128 * k : 128 * (k + 1), :])

            # ---- projections ----
            # Inputs staged as [128, 512] quarter-tiles, loaded in the exact
            # order each projection consumes them (n-outer loops), so compute
            # starts ~2 MB into the DMA stream and never re-starves.
            IQ = 512
            NQ = S // IQ

            def _ld(dram, k, qq, split=False):
                tq = ip.tile([128, IQ], bf16, tag="inq",
                             name=f"in_{dram.name}_{k}_{qq}")
                if split:  # two partition-half DMAs -> two queues in parallel
                    nc.sync.dma_start(
                        tq[0:64, :],
                        dram[128 * k : 128 * k + 64, IQ * qq : IQ * (qq + 1)],
                    )
                    nc.sync.dma_start(
                        tq[64:128, :],
                        dram[128 * k + 64 : 128 * (k + 1), IQ * qq : IQ * (qq + 1)],
                    )
                else:
                    nc.sync.dma_start(
                        tq[:], dram[128 * k : 128 * (k + 1), IQ * qq : IQ * (qq + 1)]
                    )
                return tq

            with tc.tile_pool(name="psA", bufs=8, space="PSUM") as psA:
                vt = {}
                qt = {}
                kt = {}
                for k in range(KD):
                    vt[k, 0] = _ld(vT, k, 0)
                for k in range(KD):
                    nc.sync.dma_start(wv_sb[:, k, :], wv[128 * k : 128 * (k + 1), :])
                if NQ > 1:
                    for k in range(KD):
                        vt[k, 1] = _ld(vT, k, 1)
                for k in range(KD):
                    nc.sync.dma_start(wq_sb[:, k, :], wq[128 * k : 128 * (k + 1), :])

                # v projection: v[s, e] = values @ Wv.T (natural layout)
                for qq in range(NQ):
                    if qq + 2 < NQ:
                        for k in range(KD):
                            vt[k, qq + 2] = _ld(vT, k, qq + 2)
                    for k in range(KD):
                        qt[k, qq] = _ld(qT, k, qq)
                    if qq == NQ - 1:
                        for k in range(KD):
                            nc.sync.dma_start(
                                wk_sb[:, k, :], wk[128 * k : 128 * (k + 1), :]
                            )
                        for k in range(MB):
                            nc.sync.dma_start(
                                wo_sb[:, k, :], wo[128 * k : 128 * (k + 1), :]
                            )
                    for sb in range(4 * qq, 4 * (qq + 1)):
                        ps = psA.tile([128, EC], f32, tag="ps", name=f"vp{sb}")
                        for k in range(KD):
                            nc.tensor.matmul(
                                ps[:, :],
                                vt[k, qq][:, 128 * (sb % 4) : 128 * (sb % 4 + 1)],
                                wv_sb[:, k, :],
                                start=(k == 0),
                                stop=(k == KD - 1),
                            )
                        nc.scalar.activation(
                            v_sb[:, sb, :, 0:E],
                            ps[:, :].rearrange("p (h e) -> p h e", h=HC),
                            mybir.ActivationFunctionType.Copy,
                        )

                # q projection: q^T[e, l] (+ per-partition bias bq)
                for n in range(NQ):
                    for k in range(KD):
                        kt[k, n] = _ld(kT, k, n)
                    for m in range(MB):
                        ps = psA.tile([128, 512], f32, tag="ps", name=f"qp{m}_{n}")
                        for k in range(KD):
                            nc.tensor.matmul(
                                ps[:, :],
                                wq_sb[:, k, 128 * m : 128 * (m + 1)],
                                qt[k, n][:, :],
                                start=(k == 0),
                                stop=(k == KD - 1),
                            )
                        nc.vector.tensor_scalar_add(
                            q_sb[:, m, 512 * n : 512 * (n + 1)],
                            ps[:, :],
                            bq_t[:, m : m + 1],
                        )

            # ---- attention (k-projection interleaved) ----
            with (
                tc.tile_pool(name="psS", bufs=2, space="PSUM") as psS,
                tc.tile_pool(name="psO", bufs=2, space="PSUM") as psO,
            ):
                def k_proj(nrange):
                    # zero-padded per-head k^T[e, s] (no bias); shares the
                    # "sps" PSUM slots with the attention pipeline
                    for n in nrange:
                        for m in range(MB):
                            ps = psS.tile([128, 512], f32, tag="sps",
                                          name=f"kp{m}_{n}")
                            for k in range(KD):
                                nc.tensor.matmul(
                                    ps[:, :],
                                    wk_sb[:, k, 128 * m : 128 * (m + 1)],
                                    kt[k, n][:, :],
                                    start=(k == 0),
                                    stop=(k == KD - 1),
                                )
                            for hh in range(HPB):
                                nc.vector.tensor_copy(
                                    k_sb[hh * E : (hh + 1) * E,
                                         HPB * m + hh, 512 * n : 512 * (n + 1)],
                                    ps[hh * E : (hh + 1) * E, :],
                                )

                # k columns for attention chunk j are [0, CH*(j+1)) = quarters
                # [0, (j+1)*CH//IQ); emit just-enough k-proj before each chunk
                kq_done = 0
                for j in range(NJ):
                    kq_need = min(NQ, -(-((j + 1) * CH) // IQ))
                    k_proj(range(kq_done, kq_need))
                    kq_done = kq_need
                    for h in range(HC):
                        m = h // HPB
                        poff = (h % HPB) * E
                        ops = psO.tile([128, CH], f32, tag="ops",
                                       name=f"ops{h}_{j}")
                        nsb = SBC * (j + 1)  # s-blocks touching this chunk
                        for i in range(nsb):
                            col0 = max(0, 128 * i - CH * j)
                            bl = []
                            c = col0
                            while c < CH:
                                n = min(512 - (c % 512), CH - c)
                                bl.append((c, n))
                                c += n
                            sps = psS.tile([128, CH], f32, tag="sps",
                                           name=f"sps{h}_{i}_{j}")
                            for (c, n) in bl:
                                nc.tensor.matmul(
                                    sps[:, c : c + n],
                                    k_sb[:, h, 128 * i : 128 * (i + 1)],
                                    q_sb[:, m, CH * j + c : CH * j + c + n],
                                    start=True,
                                    stop=True,
                                )
                            p_t = kp.tile([128, CH], bf16, tag="p_t",
                                          name=f"pt{h}_{i}_{j}")
                            nc.scalar.activation(
                                p_t[:, col0:CH],
                                sps[:, col0:CH],
                                mybir.ActivationFunctionType.Exp,
                                scale=scale,
                            )
                            if i >= SBC * j:  # diagonal block: causal mask
                                nc.vector.tensor_mul(
                                    p_t[:, col0 : col0 + 128],
                                    p_t[:, col0 : col0 + 128],
                                    tri[:, :],
                                )
                            for (c, n) in bl:
                                be = min(CH, 512 * (c // 512) + 512)
                                i_last = (be + CH * j) // 128 - 1
                                nc.tensor.matmul(
                                    ops[0 : E + 1, c : c + n],
                                    v_sb[:, i, h, :],
                                    p_t[:, c : c + n],
                                    start=(i == 0),
                                    stop=(i == i_last),
                                )
                        # softmax denominator: PSUM row E=64 of ops
                        rs = dp.tile([128, CH], f32, tag="rs", bufs=2)
                        nc.vector.tensor_copy(rs[E : E + 1, :], ops[E : E + 1, :])
                        rs0 = dp.tile([128, CH], f32, tag="rs0", bufs=2)
                        # custom GpSimd/DVE ops need base-partition-0 operands
                        # on HW: DMA the sum row to partition 0, broadcast it,
                        # then take the approx reciprocal on the [0:E] block.
                        nc.sync.dma_start(rs0[0:1, :], rs[E : E + 1, :])
                        rr = dp.tile([128, CH], f32, tag="rr")
                        nc.gpsimd.partition_broadcast(
                            rr[0:E, :], rs0[0:1, :], channels=E
                        )
                        nc.vector.reciprocal_approx_fast(rr[0:E, :], rr[0:E, :])
                        o_tmp = dp.tile([128, CH], bf16, tag="o_tmp")
                        nc.vector.tensor_mul(
                            o_tmp[0:E, :], ops[0:E, :], rr[0:E, :]
                        )
                        nc.sync.dma_start(
                            o_sb[poff : poff + E, m, CH * j : CH * (j + 1)],
                            o_tmp[0:E, :],
                        )

                # ---- output projection (shares the "sps" PSUM slots so its
                # matmuls backfill the tail of the attention pipeline) ----
                DC = min(512, D)
                NDC = D // DC
                for lb in range(LB):
                    for dc in range(NDC):
                        pso = psS.tile([128, DC], f32, tag="sps",
                                       name=f"op{lb}_{dc}")
                        for k in range(MB):
                            nc.tensor.matmul(
                                pso[:, :],
                                o_sb[:, k, 128 * lb : 128 * (lb + 1)],
                                wo_sb[:, k, DC * dc : DC * (dc + 1)],
                                start=(k == 0),
                                stop=(k == MB - 1),
                            )
                        ot = kp.tile([128, DC], f32, tag="ot", name=f"ot{lb}_{dc}")
                        nc.vector.tensor_copy(ot[:, :], pso[:, :])
                        nc.sync.dma_start(
                            out[128 * lb : 128 * (lb + 1), DC * dc : DC * (dc + 1)],
                            ot[:, :],
                        )

            if dump:
                dq = nc.dram_tensor("dq", [128, MB, L], bf16, kind="ExternalOutput")
                dk = nc.dram_tensor("dk", [128, HC, S], bf16, kind="ExternalOutput")
                dv = nc.dram_tensor(
                    "dv", [128, S // 128, HC, E + 1], bf16, kind="ExternalOutput"
                )
                do = nc.dram_tensor("do", [128, MB, L], bf16, kind="ExternalOutput")
                nc.sync.dma_start(dq[:], q_sb[:])
                nc.sync.dma_start(dk[:], k_sb[:])
                nc.sync.dma_start(dv[:], v_sb[:])
                nc.sync.dma_start(do[:], o_sb[:])


    nc.compile()
    return nc


def _prep_inputs(queries, keys, values, Wq, bq, Wk, Wv, Wo):
    """Build the 8 per-core input maps (host-side shard + transpose + cast)."""
    MB = EC // 128
    in_maps = []
    qT = [np.ascontiguousarray(queries[b].T.astype(_BF16)) for b in range(B)]
    kT = [np.ascontiguousarray(keys[b].T.astype(_BF16)) for b in range(B)]
    vT = [np.ascontiguousarray(values[b].T.astype(_BF16)) for b in range(B)]
    wqs, wks, wvs, wos, bqs = [], [], [], [], []
    for g in range(GROUPS):
        sl = slice(g * EC, (g + 1) * EC)
        wqs.append(np.ascontiguousarray(Wq[sl, :].T.astype(_BF16)))
        wks.append(np.ascontiguousarray(Wk[sl, :].T.astype(_BF16)))
        wvs.append(np.ascontiguousarray(Wv[sl, :].T.astype(_BF16)))
        wos.append(np.ascontiguousarray(Wo[:, sl].T.astype(_BF16)))
        bqs.append(
            np.ascontiguousarray(
                bq[sl].astype(np.float32).reshape(MB, 128).T
            )
        )
    for c in range(NCORES):
        b, g = c // GROUPS, c % GROUPS
        in_maps.append(
            {
                "qT": qT[b], "kT": kT[b], "vT": vT[b],
                "wq": wqs[g], "wk": wks[g], "wv": wvs[g],
                "wo": wos[g], "bq": bqs[g],
            }
        )
    return in_maps


_NC_CACHE = {}


def kernel(queries, keys, values, attn_mask, Wq, bq, Wk, bk, Wv, bv, Wo, bo,
           _trace=False):
    from concourse.bass_utils import run_bass_kernel_spmd

    queries = np.asarray(queries, np.float32)
    keys = np.asarray(keys, np.float32)
    values = np.asarray(values, np.float32)
    Wq, Wk, Wv, Wo = (np.asarray(a, np.float32) for a in (Wq, Wk, Wv, Wo))
    bq, bk, bv, bo = (np.asarray(a, np.float32) for a in (bq, bk, bv, bo))

    if "nc" not in _NC_CACHE:
        _NC_CACHE["nc"] = build()
    nc = _NC_CACHE["nc"]

    in_maps = _prep_inputs(queries, keys, values, Wq, bq, Wk, Wv, Wo)
    res = run_bass_kernel_spmd(
        nc, in_maps, core_ids=list(range(NCORES)), trace=_trace
    )
    _NC_CACHE["last_results"] = res

    out = np.zeros((B, L, D), np.float32)
    for c in range(NCORES):
        out[c // GROUPS] += res.results[c]["out"]
    # bv exits through the (row-sum-1) softmax as Wo @ bv; bo is direct.
    out += (Wo @ bv + bo)[None, None, :]
    return out
